# revision 19
# baseline (speedup 1.0000x reference)
"""MoE layer (B=8192, D=1024, E=8, top-2, H=2048) on 8 TRN2 NeuronCores.

Strategy (expert-parallel, mixed fp8/bf16 precision):
  - Host: gate logits = x @ Wg (fp32), exact top-2 (jax tie-break semantics),
    softmax over the 2 picks.
  - Precision split: each (token, expert) pair's MLP runs either in bf16 or
    in fp8-e4m3 with DoubleRow matmuls (2x PE throughput). fp8 noise on a
    pair is damped by its gate weight g in the combine, so the pairs with
    the SMALLEST gates go to fp8, subject to a total output-error budget
    (err^2 ~ ERR_COEF * sum over fp8 pairs of g^2, calibrated vs the
    reference).
  - Load balance: every expert contributes exactly NB pairs to bf16 (its
    largest-gate pairs); core c runs expert c's whole bf16 pool -> the
    expensive bf16 phase is perfectly balanced with ONE weight set per
    core. All routing variance lands in the cheap fp8 family, which is cut
    into two fixed-size segments per core (pieces of <=2 experts each, same
    scheme as the bf16 planner in the previous revision).
  - Device (SPMD): phases per core: bf16 chunks, then fp8 seg a, fp8 seg b.
    bf16 runs first (the DMA ramp only has to feed one 4MB weight set, and
    the issue stream is the binding constraint early: each dma_start costs
    ~0.64us on the issuing engine); the fp8 weights stream in on the SCALAR
    engine's spare issue slots during the long bf16 phase. y = relu(x @ W1
    + b1) @ W2 with fp32 PSUM accumulation; fp8 phases emit y in bf16,
    bf16 phases in fp32. Weights resident in SBUF; tokens in column chunks
    <=512. fp8 matmuls use DoubleRow (measured ~2x bf16 throughput), and
    the small final fp8 chunk keeps the output-DMA drain short.
  - Host: weighted combine out[tok] = sum_k gate * (y + b2[e]).
"""

import os

import numpy as np
import ml_dtypes

B, D, E, TOP_K = 8192, 1024, 8, 2
H = 2 * D
P = 128
CHUNK = 512
TAIL_W = 128  # last bf16 chunk width (short output drain)

KD = D // P  # contraction tiles for mm1 (over D)
MH = H // P  # output tiles for mm1 / contraction tiles for mm2 (over H)
MD = D // P  # output tiles for mm2 (over D)

_BF16 = np.dtype(ml_dtypes.bfloat16)
_F8 = np.dtype(ml_dtypes.float8_e4m3)  # TRN fp8e4 semantics (max +-240)

# Error model, calibrated against the key-0 reference data:
#   rel_err^2 ~= BASE_ERR2 + ERR_COEF * sum_{fp8 pairs} g^2
# (bf16-everything measures 3.33e-3; full-fp8 of all g<=0.5 pairs 3.00e-2.)
BASE_ERR2 = (3.4e-3) ** 2
ERR_COEF = 7.26e-7
TARGET_ERR = 1.55e-2  # budget target; tolerance is 2e-2 (measured tracks model)

LAST_RESULTS = None  # BassKernelResults of the most recent run (for test harness)


def _chunk_sizes(n):
    """Split n columns into matmul chunks <=512, avoiding tiny tails."""
    chunks = [CHUNK] * (n // CHUNK)
    tail = n % CHUNK
    if tail >= 128 or not chunks:
        if tail:
            chunks.append(tail)
    elif tail:
        last = chunks.pop() + tail
        chunks += [last - last // 2, last // 2]
    return chunks


def _plan_segments(counts):
    """Pick segment sizes (S1 >= S2) and cut experts into 8 S1-pieces and
    8 S2-pieces (one of each per core). Returns (S1, S2, pieces) where
    pieces[core] = [(expert, start, fill_len), (expert, start, fill_len)]
    for the S1 and S2 segment respectively. `start` indexes that expert's
    own (class-local) pair list."""
    counts = np.asarray(counts)
    order = np.argsort(-counts, kind="stable")
    best = None
    for k in range(0, E // 2 + 1):
        big = order[:k]
        small = order[E - k :] if k else order[:0]
        mid = order[k : E - k]
        S1 = max((int(-(-counts[e] // 2)) for e in big), default=0)
        S2 = max((int(-(-counts[e] // 2)) for e in small), default=0)
        if len(mid):
            if S1 == 0:  # k == 0: all experts are (S1, S2)
                S1 = int(-(-counts[mid].max() // 2))
            S2 = max(S2, int(counts[mid].max()) - S1)
        C = S1 + S2
        if best is None or C < best[0]:
            best = (C, k, S1, S2)
    _, k, S1, S2 = best
    S1 = int(-(-S1 // 16) * 16)  # 16-align (DoubleRow AP stride + DMA)
    S2 = int(-(-S2 // 16) * 16)

    big = order[:k]
    small = order[E - k :] if k else order[:0]
    mid = order[k : E - k]
    s1_pieces = []  # (expert, start, len)
    s2_pieces = []
    for e in big:
        n = int(counts[e])
        h1 = -(-n // 2)
        s1_pieces += [(int(e), 0, h1), (int(e), h1, n - h1)]
    for e in small:
        n = int(counts[e])
        h1 = -(-n // 2)
        s2_pieces += [(int(e), 0, h1), (int(e), h1, n - h1)]
    for e in mid:
        n = int(counts[e])
        a = min(n, S1)
        s1_pieces.append((int(e), 0, a))
        s2_pieces.append((int(e), a, n - a))
    assert len(s1_pieces) == E and len(s2_pieces) == E
    assert all(ln <= S1 for _, _, ln in s1_pieces)
    assert all(ln <= S2 for _, _, ln in s2_pieces)
    pieces = [[s1_pieces[i], s2_pieces[i]] for i in range(E)]
    return S1, S2, pieces


def _build_program(SF1, SF2, NB):
    import concourse.bacc as bacc
    import concourse.mybir as mybir
    import concourse.tile as tile
    from concourse.bass import ts

    SFT = SF1 + SF2
    DR = mybir.MatmulPerfMode.DoubleRow
    nc = bacc.Bacc("TRN2", target_bir_lowering=False, debug=False)
    bf16 = mybir.dt.bfloat16
    f8 = mybir.dt.float8e4
    f32 = mybir.dt.float32

    xf_d = nc.dram_tensor("xf", (D, SFT), f8, kind="ExternalInput").ap()
    xb_d = nc.dram_tensor("xb", (D, NB), bf16, kind="ExternalInput").ap()
    w1f_ds, w2f_ds = [], []
    for s in ("a", "b"):
        w1f_ds.append(nc.dram_tensor(f"w1f{s}", (D, H), f8, kind="ExternalInput").ap())
        w2f_ds.append(nc.dram_tensor(f"w2f{s}", (H, D), f8, kind="ExternalInput").ap())
    w1b_d = nc.dram_tensor("w1b", (D, H), bf16, kind="ExternalInput").ap()
    w2b_d = nc.dram_tensor("w2b", (H, D), bf16, kind="ExternalInput").ap()
    b1_d = nc.dram_tensor("b1all", (P, 3 * MH), f32, kind="ExternalInput").ap()
    ytf_d = nc.dram_tensor("ytf", (D, SFT), bf16, kind="ExternalOutput").ap()
    ytb_d = nc.dram_tensor("ytb", (D, NB), f32, kind="ExternalOutput").ap()

    xf_r = xf_d.rearrange("(ko p) c -> p ko c", p=P)
    xb_r = xb_d.rearrange("(ko p) c -> p ko c", p=P)
    w1f_rs = [w.rearrange("(ko p) h -> p ko h", p=P) for w in w1f_ds]
    w2f_rs = [w.rearrange("(ko p) d -> p ko d", p=P) for w in w2f_ds]
    w1b_r = w1b_d.rearrange("(ko p) h -> p ko h", p=P)
    w2b_r = w2b_d.rearrange("(ko p) d -> p ko d", p=P)

    with tile.TileContext(nc) as tc:
        with (
            tc.tile_pool(name="weights", bufs=1) as wpool,
            tc.tile_pool(name="xin", bufs=1) as xpool,
            tc.tile_pool(name="hbuf", bufs=1) as hpool,
            tc.tile_pool(name="ystage", bufs=4) as ypool,
            tc.tile_pool(name="ps", bufs=8, space="PSUM") as pspool,
        ):
            fw1 = wpool.tile([P, 2, KD, H], f8, name="fw1")
            fw2 = wpool.tile([P, 2, MH, D], f8, name="fw2")
            bw1 = wpool.tile([P, KD, H], bf16, name="bw1")
            bw2 = wpool.tile([P, MH, D], bf16, name="bw2")
            b1_sb = wpool.tile([P, 3, MH], f32, name="b1_sb")
            xf_sb = xpool.tile([P, KD, SFT], f8, name="xf_sb")
            xb_sb = xpool.tile([P, KD, NB], bf16, name="xb_sb")

            # ---- DMA ramp (sync engine). Issue order == need order; each
            # dma_start costs ~0.64us of issue time, each piece lands on one
            # queue (~27GB/s), so piece sizes are matched to when the bf16
            # chunk-0 loop consumes them. ----
            nc.sync.dma_start(b1_sb, b1_d.rearrange("p (s m) -> p s m", s=3))
            xb_c0 = min(CHUNK, NB)
            for k in range(0, KD, 2):  # bf16 w1 m-group 0: 4 x 64KB
                nc.sync.dma_start(
                    bw1[:, k : k + 2, 0:128], w1b_r[:, k : k + 2, 0:128]
                )
            for k in range(KD):  # x bf16 chunk 0: 8 x 128KB
                nc.sync.dma_start(xb_sb[:, k, 0:xb_c0], xb_r[:, k, 0:xb_c0])
            for k in range(0, KD, 2):  # bf16 w1 m-groups 1..3: 4 x 192KB
                nc.sync.dma_start(
                    bw1[:, k : k + 2, 128:512], w1b_r[:, k : k + 2, 128:512]
                )
            for k in range(0, MH, 2):  # bf16 w2: 8 x 256KB
                nc.sync.dma_start(bw2[:, k : k + 2], w2b_r[:, k : k + 2])
            if NB > xb_c0:
                for k in range(0, KD, 2):  # x bf16 remaining cols
                    nc.sync.dma_start(
                        xb_sb[:, k : k + 2, xb_c0:NB], xb_r[:, k : k + 2, xb_c0:NB]
                    )

            # The scalar engine (also HWDGE) carries a second issue stream:
            # the bw1 bulk lands just-in-time for mm1 chunk-0's m4..15 while
            # sync is still busy issuing the x / w2 pieces.
            for k in range(0, KD, 2):  # m-groups 4..5: 4 x 128KB
                nc.scalar.dma_start(
                    bw1[:, k : k + 2, 512:768], w1b_r[:, k : k + 2, 512:768]
                )
            for k in range(0, KD, 2):  # m-groups 6..9: 4 x 256KB
                nc.scalar.dma_start(
                    bw1[:, k : k + 2, 768:1280], w1b_r[:, k : k + 2, 768:1280]
                )
            for k in range(0, KD, 2):  # m-groups 10..15: 4 x 384KB
                nc.scalar.dma_start(
                    bw1[:, k : k + 2, 1280:2048], w1b_r[:, k : k + 2, 1280:2048]
                )

            def emit_f8_weight_dmas(part):
                # Scalar-issued mid-kernel; lands long before the fp8 phases.
                if part == 0:
                    for k in range(0, KD, 2):  # x fp8
                        nc.scalar.dma_start(xf_sb[:, k : k + 2], xf_r[:, k : k + 2])
                    for k in range(0, KD, 2):  # fp8 seg-a w1: 4 x 512KB
                        nc.scalar.dma_start(
                            fw1[:, 0, k : k + 2], w1f_rs[0][:, k : k + 2]
                        )
                    for k in range(0, MH, 4):  # fp8 seg-a w2: 4 x 512KB
                        nc.scalar.dma_start(
                            fw2[:, 0, k : k + 4], w2f_rs[0][:, k : k + 4]
                        )
                else:
                    for k in range(0, KD, 2):
                        nc.scalar.dma_start(
                            fw1[:, 1, k : k + 2], w1f_rs[1][:, k : k + 2]
                        )
                    for k in range(0, MH, 4):
                        nc.scalar.dma_start(
                            fw2[:, 1, k : k + 4], w2f_rs[1][:, k : k + 4]
                        )

            # ---- chunk list: bf16 chunks, then fp8 seg a / seg b last ----
            chunk_list = []  # (kind, seg, off, tw)   off: class-local column
            off = 0
            for tw in _chunk_sizes(NB):
                chunk_list.append(("bf", 0, off, tw))
                off += tw
            off = 0
            for seg, seg_len in ((0, SF1), (1, SF2)):
                for tw in _chunk_sizes(seg_len):
                    chunk_list.append(("f8", seg, off, tw))
                    off += tw

            # PE warmup: junk matmuls on a memset tile while first DMAs land,
            # so the HAM clock gate is at 8/8 when real matmuls start.
            warm_sb = xpool.tile([P, P], bf16, name="warm")
            nc.vector.memset(warm_sb, 0.0)
            warm_ps = pspool.tile([P, P], f32, tag="ps", name="warm_ps")
            for _ in range(45):
                nc.tensor.matmul(warm_ps, warm_sb, warm_sb, start=True, stop=True)

            def mm1(kind, seg, off, tw, h_sb, fillers):
                if kind == "f8":
                    x_sb, b1_col = xf_sb, b1_sb[:, 1 + seg]
                else:
                    x_sb, b1_col = xb_sb, b1_sb[:, 0]
                for m in range(MH):
                    ph = pspool.tile([P, CHUNK], f32, tag="ps", name="ph")
                    if kind == "f8":
                        for k in range(0, KD, 2):
                            nc.tensor.matmul(
                                ph[:, :tw],
                                fw1[:, seg, k : k + 2, ts(m, P)],
                                x_sb[:, k : k + 2, off : off + tw],
                                start=(k == 0),
                                stop=(k == KD - 2),
                                perf_mode=DR,
                            )
                    else:
                        for k in range(KD):
                            nc.tensor.matmul(
                                ph[:, :tw],
                                bw1[:, k, ts(m, P)],
                                x_sb[:, k, off : off + tw],
                                start=(k == 0),
                                stop=(k == KD - 1),
                            )
                    nc.scalar.activation(
                        h_sb[:, m, :tw],
                        ph[:, :tw],
                        mybir.ActivationFunctionType.Relu,
                        bias=b1_col[:, m : m + 1],
                    )
                    if fillers and m < 8:
                        # dependency-free matmuls keep the PE clock gate at
                        # 8/8 while chunk-0 weights stream in
                        for _ in range(5):
                            nc.tensor.matmul(
                                warm_ps, warm_sb, warm_sb, start=True, stop=True
                            )

            def mm2(kind, seg, off, tw, h_sb, k2_outer):
                # k2-outer: all psum banks accumulate together so each w2[k2]
                # slice is consumed as it lands. Last chunk uses m2-outer so
                # its copies/output DMAs overlap its own matmul stream.
                ydt = bf16 if kind == "f8" else f32
                yt_d = ytf_d if kind == "f8" else ytb_d

                def one_mm2(py, m2, k2, start, stop):
                    if kind == "f8":
                        nc.tensor.matmul(
                            py[:, :tw],
                            fw2[:, seg, k2 : k2 + 2, ts(m2, P)],
                            h_sb[:, k2 : k2 + 2, :tw],
                            start=start,
                            stop=stop,
                            perf_mode=DR,
                        )
                    else:
                        nc.tensor.matmul(
                            py[:, :tw],
                            bw2[:, k2, ts(m2, P)],
                            h_sb[:, k2, :tw],
                            start=start,
                            stop=stop,
                        )

                kstep = 2 if kind == "f8" else 1
                if k2_outer:
                    for m2_base in (0, MD // 2):
                        m2s = range(m2_base, m2_base + MD // 2)
                        pys = {
                            m2: pspool.tile([P, CHUNK], f32, tag="ps", name=f"py{m2}")
                            for m2 in m2s
                        }
                        for k2 in range(0, MH, kstep):
                            for m2 in m2s:
                                one_mm2(
                                    pys[m2], m2, k2, k2 == 0, k2 == MH - kstep
                                )
                        for m2 in m2s:
                            y_sb = ypool.tile([P, CHUNK], ydt, tag="y", name="y_sb")
                            nc.vector.tensor_copy(y_sb[:, :tw], pys[m2][:, :tw])
                            nc.sync.dma_start(
                                yt_d[ts(m2, P), off : off + tw], y_sb[:, :tw]
                            )
                else:
                    # last chunk: m2-outer, outputs paired into one DMA per
                    # two m2 tiles so the post-matmul drain is issue-light
                    yt_r = yt_d.rearrange("(m p) c -> p m c", p=P)
                    for m2_base in range(0, MD, 2):
                        y2_sb = ypool.tile(
                            [P, 2, CHUNK], ydt, tag="y2", bufs=4, name="y2_sb"
                        )
                        for j in (0, 1):
                            m2 = m2_base + j
                            py = pspool.tile([P, CHUNK], f32, tag="ps", name="py")
                            for k2 in range(0, MH, kstep):
                                one_mm2(py, m2, k2, k2 == 0, k2 == MH - kstep)
                            nc.vector.tensor_copy(y2_sb[:, j, :tw], py[:, :tw])
                        nc.sync.dma_start(
                            yt_r[:, m2_base : m2_base + 2, off : off + tw],
                            y2_sb[:, :, :tw],
                        )

            nb_chunks = sum(1 for c in chunk_list if c[0] == "bf")
            for ci, (kind, seg, off, tw) in enumerate(chunk_list):
                hdt = f8 if kind == "f8" else bf16
                h_sb = hpool.tile([P, MH, CHUNK], hdt, tag="h", name="h_sb")
                mm1(kind, seg, off, tw, h_sb, fillers=(ci == 0))
                if ci == 0:
                    emit_f8_weight_dmas(0)
                if ci == min(1, nb_chunks - 1):
                    emit_f8_weight_dmas(1)
                mm2(kind, seg, off, tw, h_sb, k2_outer=(ci < len(chunk_list) - 1))
    nc.finalize()
    return nc


def _route(x, Wg):
    """Exact reference gating on host: top-2 of clean fp32 logits (jax
    tie-break: lower index first), softmax over the two picks."""
    logits = x @ Wg  # [B, E] fp32
    order = np.argsort(-logits, axis=1, kind="stable")[:, :TOP_K]  # [B, 2]
    top_vals = np.take_along_axis(logits, order, axis=1)
    ex = np.exp(top_vals - top_vals[:, :1])  # top_vals sorted desc -> max first
    gates = (ex / ex.sum(axis=1, keepdims=True)).astype(np.float32)  # [B, 2]
    return order, gates


def kernel(x, Wg, W1, b1, W2, b2):
    x = np.ascontiguousarray(np.asarray(x, dtype=np.float32))
    Wg = np.asarray(Wg, dtype=np.float32)
    W1 = np.asarray(W1, dtype=np.float32)
    b1 = np.asarray(b1, dtype=np.float32)
    W2 = np.asarray(W2, dtype=np.float32)
    b2 = np.asarray(b2, dtype=np.float32)

    order, gates = _route(x, Wg)

    # Flatten (token, k) pairs, bucket by expert (stable => slot order within
    # an expert follows token order). Pair p belongs to token p//2.
    expert_flat = order.reshape(-1)  # [2B]
    gate_flat = gates.reshape(-1)  # [2B]
    perm = np.argsort(expert_flat, kind="stable")  # pairs grouped by expert
    counts = np.bincount(expert_flat, minlength=E)
    offs = np.concatenate(([0], np.cumsum(counts)))[:E]

    # ---- precision split: per expert, the NB largest-gate pairs run bf16;
    # the rest run fp8. NB is the smallest uniform bf16 pool size whose
    # predicted total error stays within budget. ----
    by_exp = [perm[offs[e] : offs[e] + counts[e]] for e in range(E)]
    g_by_exp = [np.sort(gate_flat[idx]) for idx in by_exp]
    cumg2 = [np.cumsum(g.astype(np.float64) ** 2) for g in g_by_exp]
    budget = TARGET_ERR**2 - BASE_ERR2

    def err2(NB):
        s = 0.0
        for e in range(E):
            nf = counts[e] - NB
            if nf > 0:
                s += cumg2[e][nf - 1]
        return ERR_COEF * s

    NB = int(min(counts))
    lo, hi = 1, int(min(counts))  # find smallest NB with err2 <= budget
    while lo < hi:
        mid = (lo + hi) // 2
        if err2(mid) <= budget:
            hi = mid
        else:
            lo = mid + 1
    NB = lo
    NB += -NB % 2  # keep bf16 x columns 4B-aligned
    NB = min(NB, int(min(counts)))
    nf_counts = counts - NB

    SF1, SF2, piecesF = _plan_segments(nf_counts)
    SFT = SF1 + SF2
    pred_err = float(np.sqrt(BASE_ERR2 + err2(NB)))

    # Per-expert class-local pair lists (token order): fp8 = smallest gates.
    f8_pairs, bf_pairs = [], []
    for e in range(E):
        idx = by_exp[e]
        ge = gate_flat[idx]
        nf = int(nf_counts[e])
        if nf > 0:
            thresh = np.partition(ge, nf - 1)[nf - 1]
            isf8 = ge < thresh
            need = nf - int(isf8.sum())  # break ties at the threshold
            at = np.nonzero(ge == thresh)[0]
            isf8[at[:need]] = True
        else:
            isf8 = np.zeros(len(idx), dtype=bool)
        f8_pairs.append(idx[isf8])
        bf_pairs.append(idx[~isf8])

    # Placement maps for the combine step.
    core_of_pair = np.empty(2 * B, dtype=np.int64)
    col_of_pair = np.empty(2 * B, dtype=np.int64)
    isf8_pair = np.zeros(2 * B, dtype=bool)

    xT = np.ascontiguousarray(x.T)  # [D, B] f32
    xT8 = xT.astype(_F8)
    f8_w1 = [None] * E
    f8_w2 = [None] * E
    bf_w1 = [None] * E
    bf_w2 = [None] * E

    in_maps = []
    for core in range(E):
        xf = np.zeros((D, SFT), dtype=_F8)
        in_map = {"xf": xf}
        # fp8 segments
        for seg, (e, start, ln) in enumerate(piecesF[core]):
            seg_off = 0 if seg == 0 else SF1
            if ln:
                pidx = f8_pairs[e][start : start + ln]
                toks = pidx // 2
                xf[:, seg_off : seg_off + ln] = xT8[:, toks]
                core_of_pair[pidx] = core
                col_of_pair[pidx] = seg_off + np.arange(ln)
                isf8_pair[pidx] = True
            if f8_w1[e] is None:
                f8_w1[e] = W1[e].astype(_F8)
                f8_w2[e] = W2[e].astype(_F8)
            s = "ab"[seg]
            in_map[f"w1f{s}"] = f8_w1[e]
            in_map[f"w2f{s}"] = f8_w2[e]
            in_map.setdefault("_b1f", []).append(
                np.ascontiguousarray(b1[e].reshape(MH, P).T)
            )
        # bf16 segment: core c <-> expert c
        e = core
        pidx = bf_pairs[e]
        assert len(pidx) == NB or len(pidx) == counts[e]
        xb = np.zeros((D, NB), dtype=_BF16)
        toks = pidx // 2
        xb[:, : len(pidx)] = xT[:, toks].astype(_BF16)
        core_of_pair[pidx] = core
        col_of_pair[pidx] = np.arange(len(pidx))
        if bf_w1[e] is None:
            bf_w1[e] = W1[e].astype(_BF16)
            bf_w2[e] = W2[e].astype(_BF16)
        in_map["xb"] = xb
        in_map["w1b"] = bf_w1[e]
        in_map["w2b"] = bf_w2[e]
        b1b_col = np.ascontiguousarray(b1[e].reshape(MH, P).T)
        in_map["b1all"] = np.ascontiguousarray(
            np.concatenate([b1b_col] + in_map.pop("_b1f"), axis=1)
        )
        in_maps.append(in_map)

    nc = _build_program(SF1, SF2, NB)

    from concourse.bass_utils import run_bass_kernel_spmd

    trace = os.environ.get("MOE_TRACE") == "1"
    kwargs = {}
    if trace:
        kwargs = dict(trace=True, trace_cores=list(range(E)))
    try:
        res = run_bass_kernel_spmd(nc, in_maps, core_ids=list(range(E)), **kwargs)
    except Exception:  # wedged accelerator: reset once and retry untraced
        try:
            import ctypes

            lib = ctypes.CDLL("/opt/axon/libaxon_pjrt.so")
            lib.axon_reset.restype = ctypes.c_int64
            lib.axon_reset()
        except OSError:
            pass
        res = run_bass_kernel_spmd(nc, in_maps, core_ids=list(range(E)))
    global LAST_RESULTS
    LAST_RESULTS = res
    LAST_RESULTS.pred_err = pred_err

    YF = np.stack([np.asarray(r["ytf"], dtype=np.float32) for r in res.results])
    YB = np.stack([np.asarray(r["ytb"], dtype=np.float32) for r in res.results])

    # Combine: pair p contributes gate_p * (y[:, col_p] + b2[e_p]) to token
    # p//2. Pairs of token b sit at flat positions 2b, 2b+1.
    cols = np.empty((2 * B, D), dtype=np.float32)
    m8 = isf8_pair
    cols[m8] = YF[core_of_pair[m8], :, col_of_pair[m8]]
    cols[~m8] = YB[core_of_pair[~m8], :, col_of_pair[~m8]]
    weighted = (cols + b2[expert_flat]) * gate_flat[:, None]
    out = weighted[0::2] + weighted[1::2]
    return np.ascontiguousarray(out, dtype=np.float32)


# revision 21
# speedup vs baseline: 1.0192x; 1.0192x over previous
"""MoE layer (B=8192, D=1024, E=8, top-2, H=2048) on 8 TRN2 NeuronCores.

Strategy (expert-parallel, mixed fp8/bf16 precision):
  - Host: gate logits = x @ Wg (fp32), exact top-2 (jax tie-break semantics),
    softmax over the 2 picks.
  - Precision split: each (token, expert) pair's MLP runs either in bf16 or
    in fp8-e4m3 with DoubleRow matmuls (2x PE throughput). fp8 noise on a
    pair is damped by its gate weight g in the combine, so the pairs with
    the SMALLEST gates go to fp8, subject to a total output-error budget
    (err^2 ~ ERR_COEF * sum over fp8 pairs of g^2, calibrated vs the
    reference).
  - Load balance: every expert contributes exactly NB pairs to bf16 (its
    largest-gate pairs); core c runs expert c's whole bf16 pool -> the
    expensive bf16 phase is perfectly balanced with ONE weight set per
    core. All routing variance lands in the cheap fp8 family, which is cut
    into two fixed-size segments per core (pieces of <=2 experts each, same
    scheme as the bf16 planner in the previous revision).
  - Device (SPMD): phases per core: bf16 chunks, then fp8 seg a, fp8 seg b.
    bf16 runs first (the DMA ramp only has to feed one 4MB weight set, and
    the issue stream is the binding constraint early: each dma_start costs
    ~0.64us on the issuing engine); the fp8 weights stream in on the SCALAR
    engine's spare issue slots during the long bf16 phase. y = relu(x @ W1
    + b1) @ W2 with fp32 PSUM accumulation; fp8 phases emit y in bf16,
    bf16 phases in fp32. Weights resident in SBUF; tokens in column chunks
    <=512. fp8 matmuls use DoubleRow (measured ~2x bf16 throughput), and
    the small final fp8 chunk keeps the output-DMA drain short.
  - Host: weighted combine out[tok] = sum_k gate * (y + b2[e]).
"""

import os

import numpy as np
import ml_dtypes

B, D, E, TOP_K = 8192, 1024, 8, 2
H = 2 * D
P = 128
CHUNK = 512
TAIL_W = 128  # last bf16 chunk width (short output drain)

KD = D // P  # contraction tiles for mm1 (over D)
MH = H // P  # output tiles for mm1 / contraction tiles for mm2 (over H)
MD = D // P  # output tiles for mm2 (over D)

_BF16 = np.dtype(ml_dtypes.bfloat16)
_F8 = np.dtype(ml_dtypes.float8_e4m3)  # TRN fp8e4 semantics (max +-240)

# Error model, calibrated against the key-0 reference data:
#   rel_err^2 ~= BASE_ERR2 + ERR_COEF * sum_{fp8 pairs} g^2
# (bf16-everything measures 3.33e-3; full-fp8 of all g<=0.5 pairs 3.00e-2.)
BASE_ERR2 = (3.4e-3) ** 2
ERR_COEF = 7.26e-7
TARGET_ERR = 1.55e-2  # budget target; tolerance is 2e-2 (measured tracks model)

LAST_RESULTS = None  # BassKernelResults of the most recent run (for test harness)


def _chunk_sizes(n):
    """Split n columns into matmul chunks <=512, avoiding tiny tails."""
    chunks = [CHUNK] * (n // CHUNK)
    tail = n % CHUNK
    if tail >= 128 or not chunks:
        if tail:
            chunks.append(tail)
    elif tail:
        last = chunks.pop() + tail
        chunks += [last - last // 2, last // 2]
    return chunks


def _plan_segments(counts):
    """Pick segment sizes (S1 >= S2) and cut experts into 8 S1-pieces and
    8 S2-pieces (one of each per core). Returns (S1, S2, pieces) where
    pieces[core] = [(expert, start, fill_len), (expert, start, fill_len)]
    for the S1 and S2 segment respectively. `start` indexes that expert's
    own (class-local) pair list."""
    counts = np.asarray(counts)
    order = np.argsort(-counts, kind="stable")
    best = None
    for k in range(0, E // 2 + 1):
        big = order[:k]
        small = order[E - k :] if k else order[:0]
        mid = order[k : E - k]
        S1 = max((int(-(-counts[e] // 2)) for e in big), default=0)
        S2 = max((int(-(-counts[e] // 2)) for e in small), default=0)
        if len(mid):
            if S1 == 0:  # k == 0: all experts are (S1, S2)
                S1 = int(-(-counts[mid].max() // 2))
            S2 = max(S2, int(counts[mid].max()) - S1)
        C = S1 + S2
        if best is None or C < best[0]:
            best = (C, k, S1, S2)
    _, k, S1, S2 = best
    S1 = int(-(-S1 // 16) * 16)  # 16-align (DoubleRow AP stride + DMA)
    S2 = int(-(-S2 // 16) * 16)

    big = order[:k]
    small = order[E - k :] if k else order[:0]
    mid = order[k : E - k]
    s1_pieces = []  # (expert, start, len)
    s2_pieces = []
    for e in big:
        n = int(counts[e])
        h1 = -(-n // 2)
        s1_pieces += [(int(e), 0, h1), (int(e), h1, n - h1)]
    for e in small:
        n = int(counts[e])
        h1 = -(-n // 2)
        s2_pieces += [(int(e), 0, h1), (int(e), h1, n - h1)]
    for e in mid:
        n = int(counts[e])
        a = min(n, S1)
        s1_pieces.append((int(e), 0, a))
        s2_pieces.append((int(e), a, n - a))
    assert len(s1_pieces) == E and len(s2_pieces) == E
    assert all(ln <= S1 for _, _, ln in s1_pieces)
    assert all(ln <= S2 for _, _, ln in s2_pieces)
    pieces = [[s1_pieces[i], s2_pieces[i]] for i in range(E)]
    return S1, S2, pieces


def _build_program(SF1, SF2, NB):
    import concourse.bacc as bacc
    import concourse.mybir as mybir
    import concourse.tile as tile
    from concourse.bass import ts

    SFT = SF1 + SF2
    DR = mybir.MatmulPerfMode.DoubleRow
    nc = bacc.Bacc("TRN2", target_bir_lowering=False, debug=False)
    bf16 = mybir.dt.bfloat16
    f8 = mybir.dt.float8e4
    f32 = mybir.dt.float32

    xf_d = nc.dram_tensor("xf", (D, SFT), f8, kind="ExternalInput").ap()
    xb_d = nc.dram_tensor("xb", (D, NB), bf16, kind="ExternalInput").ap()
    w1f_ds, w2f_ds = [], []
    for s in ("a", "b"):
        w1f_ds.append(nc.dram_tensor(f"w1f{s}", (D, H), f8, kind="ExternalInput").ap())
        w2f_ds.append(nc.dram_tensor(f"w2f{s}", (H, D), f8, kind="ExternalInput").ap())
    w1b_d = nc.dram_tensor("w1b", (D, H), bf16, kind="ExternalInput").ap()
    w2b_d = nc.dram_tensor("w2b", (H, D), bf16, kind="ExternalInput").ap()
    b1_d = nc.dram_tensor("b1all", (P, 3 * MH), f32, kind="ExternalInput").ap()
    ytf_d = nc.dram_tensor("ytf", (D, SFT), bf16, kind="ExternalOutput").ap()
    ytb_d = nc.dram_tensor("ytb", (D, NB), f32, kind="ExternalOutput").ap()

    xf_r = xf_d.rearrange("(ko p) c -> p ko c", p=P)
    xb_r = xb_d.rearrange("(ko p) c -> p ko c", p=P)
    w1f_rs = [w.rearrange("(ko p) h -> p ko h", p=P) for w in w1f_ds]
    w2f_rs = [w.rearrange("(ko p) d -> p ko d", p=P) for w in w2f_ds]
    w1b_r = w1b_d.rearrange("(ko p) h -> p ko h", p=P)
    w2b_r = w2b_d.rearrange("(ko p) d -> p ko d", p=P)

    with tile.TileContext(nc) as tc:
        with (
            tc.tile_pool(name="weights", bufs=1) as wpool,
            tc.tile_pool(name="xin", bufs=1) as xpool,
            tc.tile_pool(name="hbuf", bufs=1) as hpool,
            tc.tile_pool(name="ystage", bufs=4) as ypool,
            tc.tile_pool(name="ps", bufs=8, space="PSUM") as pspool,
        ):
            fw1 = wpool.tile([P, 2, KD, H], f8, name="fw1")
            fw2 = wpool.tile([P, 2, MH, D], f8, name="fw2")
            bw1 = wpool.tile([P, KD, H], bf16, name="bw1")
            bw2 = wpool.tile([P, MH, D], bf16, name="bw2")
            b1_sb = wpool.tile([P, 3, MH], f32, name="b1_sb")
            xf_sb = xpool.tile([P, KD, SFT], f8, name="xf_sb")
            xb_sb = xpool.tile([P, KD, NB], bf16, name="xb_sb")

            # ---- DMA ramp (sync engine). Issue order == need order; each
            # dma_start costs ~0.64us of issue time, each piece lands on one
            # queue (~27GB/s), so piece sizes are matched to when the bf16
            # chunk-0 loop consumes them. ----
            nc.sync.dma_start(b1_sb, b1_d.rearrange("p (s m) -> p s m", s=3))
            xb_c0 = min(CHUNK, NB)
            for k in range(0, KD, 2):  # bf16 w1 m-group 0: 4 x 64KB
                nc.sync.dma_start(
                    bw1[:, k : k + 2, 0:128], w1b_r[:, k : k + 2, 0:128]
                )
            for k in range(KD):  # x bf16 chunk 0: 8 x 128KB
                nc.sync.dma_start(xb_sb[:, k, 0:xb_c0], xb_r[:, k, 0:xb_c0])
            for k in range(0, KD, 2):  # bf16 w1 m-groups 1..3: 4 x 192KB
                nc.sync.dma_start(
                    bw1[:, k : k + 2, 128:512], w1b_r[:, k : k + 2, 128:512]
                )
            for h0 in (512, 768):  # bf16 w1 m-groups 4..7: 8 x 128KB
                for k in range(0, KD, 2):
                    nc.sync.dma_start(
                        bw1[:, k : k + 2, h0 : h0 + 256],
                        w1b_r[:, k : k + 2, h0 : h0 + 256],
                    )
            for h0 in (1024, 1536):  # bf16 w1 m-groups 8..15: 8 x 256KB
                for k in range(0, KD, 2):
                    nc.sync.dma_start(
                        bw1[:, k : k + 2, h0 : h0 + 512],
                        w1b_r[:, k : k + 2, h0 : h0 + 512],
                    )
            for k in range(0, MH, 2):  # bf16 w2: 8 x 256KB
                nc.sync.dma_start(bw2[:, k : k + 2], w2b_r[:, k : k + 2])
            if NB > xb_c0:
                for k in range(0, KD, 2):  # x bf16 remaining cols
                    nc.sync.dma_start(
                        xb_sb[:, k : k + 2, xb_c0:NB], xb_r[:, k : k + 2, xb_c0:NB]
                    )

            def emit_f8_weight_dmas(part):
                # Scalar-issued mid-kernel; lands long before the fp8 phases.
                if part == 0:
                    for k in range(0, KD, 2):  # x fp8
                        nc.scalar.dma_start(xf_sb[:, k : k + 2], xf_r[:, k : k + 2])
                    for k in range(0, KD, 2):  # fp8 seg-a w1: 4 x 512KB
                        nc.scalar.dma_start(
                            fw1[:, 0, k : k + 2], w1f_rs[0][:, k : k + 2]
                        )
                    for k in range(0, MH, 4):  # fp8 seg-a w2: 4 x 512KB
                        nc.scalar.dma_start(
                            fw2[:, 0, k : k + 4], w2f_rs[0][:, k : k + 4]
                        )
                else:
                    for k in range(0, KD, 2):
                        nc.scalar.dma_start(
                            fw1[:, 1, k : k + 2], w1f_rs[1][:, k : k + 2]
                        )
                    for k in range(0, MH, 4):
                        nc.scalar.dma_start(
                            fw2[:, 1, k : k + 4], w2f_rs[1][:, k : k + 4]
                        )

            # ---- chunk list: bf16 chunks, then fp8 seg a / seg b last ----
            chunk_list = []  # (kind, seg, off, tw)   off: class-local column
            off = 0
            for tw in _chunk_sizes(NB):
                chunk_list.append(("bf", 0, off, tw))
                off += tw
            off = 0
            for seg, seg_len in ((0, SF1), (1, SF2)):
                for tw in _chunk_sizes(seg_len):
                    chunk_list.append(("f8", seg, off, tw))
                    off += tw

            # PE warmup: junk matmuls on a memset tile while first DMAs land,
            # so the HAM clock gate is at 8/8 when real matmuls start.
            warm_sb = xpool.tile([P, P], bf16, name="warm")
            nc.vector.memset(warm_sb, 0.0)
            warm_ps = pspool.tile([P, P], f32, tag="ps", name="warm_ps")
            for _ in range(70):
                nc.tensor.matmul(warm_ps, warm_sb, warm_sb, start=True, stop=True)

            def mm1(kind, seg, off, tw, h_sb, fillers):
                if kind == "f8":
                    x_sb, b1_col = xf_sb, b1_sb[:, 1 + seg]
                else:
                    x_sb, b1_col = xb_sb, b1_sb[:, 0]
                for m in range(MH):
                    ph = pspool.tile([P, CHUNK], f32, tag="ps", name="ph")
                    if kind == "f8":
                        for k in range(0, KD, 2):
                            nc.tensor.matmul(
                                ph[:, :tw],
                                fw1[:, seg, k : k + 2, ts(m, P)],
                                x_sb[:, k : k + 2, off : off + tw],
                                start=(k == 0),
                                stop=(k == KD - 2),
                                perf_mode=DR,
                            )
                    else:
                        for k in range(KD):
                            nc.tensor.matmul(
                                ph[:, :tw],
                                bw1[:, k, ts(m, P)],
                                x_sb[:, k, off : off + tw],
                                start=(k == 0),
                                stop=(k == KD - 1),
                            )
                    nc.scalar.activation(
                        h_sb[:, m, :tw],
                        ph[:, :tw],
                        mybir.ActivationFunctionType.Relu,
                        bias=b1_col[:, m : m + 1],
                    )
                    if fillers and m < 8:
                        # dependency-free matmuls keep the PE clock gate at
                        # 8/8 while chunk-0 weights stream in
                        for _ in range(5):
                            nc.tensor.matmul(
                                warm_ps, warm_sb, warm_sb, start=True, stop=True
                            )

            def mm2(kind, seg, off, tw, h_sb, k2_outer):
                # k2-outer: all psum banks accumulate together so each w2[k2]
                # slice is consumed as it lands. Last chunk uses m2-outer so
                # its copies/output DMAs overlap its own matmul stream.
                ydt = bf16 if kind == "f8" else f32
                yt_d = ytf_d if kind == "f8" else ytb_d

                def one_mm2(py, m2, k2, start, stop):
                    if kind == "f8":
                        nc.tensor.matmul(
                            py[:, :tw],
                            fw2[:, seg, k2 : k2 + 2, ts(m2, P)],
                            h_sb[:, k2 : k2 + 2, :tw],
                            start=start,
                            stop=stop,
                            perf_mode=DR,
                        )
                    else:
                        nc.tensor.matmul(
                            py[:, :tw],
                            bw2[:, k2, ts(m2, P)],
                            h_sb[:, k2, :tw],
                            start=start,
                            stop=stop,
                        )

                kstep = 2 if kind == "f8" else 1
                if k2_outer:
                    for m2_base in (0, MD // 2):
                        m2s = range(m2_base, m2_base + MD // 2)
                        pys = {
                            m2: pspool.tile([P, CHUNK], f32, tag="ps", name=f"py{m2}")
                            for m2 in m2s
                        }
                        for k2 in range(0, MH, kstep):
                            for m2 in m2s:
                                one_mm2(
                                    pys[m2], m2, k2, k2 == 0, k2 == MH - kstep
                                )
                        for m2 in m2s:
                            y_sb = ypool.tile([P, CHUNK], ydt, tag="y", name="y_sb")
                            nc.vector.tensor_copy(y_sb[:, :tw], pys[m2][:, :tw])
                            nc.sync.dma_start(
                                yt_d[ts(m2, P), off : off + tw], y_sb[:, :tw]
                            )
                else:
                    # last chunk: m2-outer, outputs paired into one DMA per
                    # two m2 tiles so the post-matmul drain is issue-light
                    yt_r = yt_d.rearrange("(m p) c -> p m c", p=P)
                    for m2_base in range(0, MD, 2):
                        y2_sb = ypool.tile(
                            [P, 2, CHUNK], ydt, tag="y2", bufs=4, name="y2_sb"
                        )
                        for j in (0, 1):
                            m2 = m2_base + j
                            py = pspool.tile([P, CHUNK], f32, tag="ps", name="py")
                            for k2 in range(0, MH, kstep):
                                one_mm2(py, m2, k2, k2 == 0, k2 == MH - kstep)
                            nc.vector.tensor_copy(y2_sb[:, j, :tw], py[:, :tw])
                        nc.sync.dma_start(
                            yt_r[:, m2_base : m2_base + 2, off : off + tw],
                            y2_sb[:, :, :tw],
                        )

            nb_chunks = sum(1 for c in chunk_list if c[0] == "bf")
            for ci, (kind, seg, off, tw) in enumerate(chunk_list):
                hdt = f8 if kind == "f8" else bf16
                h_sb = hpool.tile([P, MH, CHUNK], hdt, tag="h", name="h_sb")
                mm1(kind, seg, off, tw, h_sb, fillers=(ci == 0))
                if ci == 0:
                    emit_f8_weight_dmas(0)
                if ci == min(1, nb_chunks - 1):
                    emit_f8_weight_dmas(1)
                mm2(kind, seg, off, tw, h_sb, k2_outer=(ci < len(chunk_list) - 1))
    nc.finalize()
    return nc


def _route(x, Wg):
    """Exact reference gating on host: top-2 of clean fp32 logits (jax
    tie-break: lower index first), softmax over the two picks."""
    logits = x @ Wg  # [B, E] fp32
    order = np.argsort(-logits, axis=1, kind="stable")[:, :TOP_K]  # [B, 2]
    top_vals = np.take_along_axis(logits, order, axis=1)
    ex = np.exp(top_vals - top_vals[:, :1])  # top_vals sorted desc -> max first
    gates = (ex / ex.sum(axis=1, keepdims=True)).astype(np.float32)  # [B, 2]
    return order, gates


def kernel(x, Wg, W1, b1, W2, b2):
    x = np.ascontiguousarray(np.asarray(x, dtype=np.float32))
    Wg = np.asarray(Wg, dtype=np.float32)
    W1 = np.asarray(W1, dtype=np.float32)
    b1 = np.asarray(b1, dtype=np.float32)
    W2 = np.asarray(W2, dtype=np.float32)
    b2 = np.asarray(b2, dtype=np.float32)

    order, gates = _route(x, Wg)

    # Flatten (token, k) pairs, bucket by expert (stable => slot order within
    # an expert follows token order). Pair p belongs to token p//2.
    expert_flat = order.reshape(-1)  # [2B]
    gate_flat = gates.reshape(-1)  # [2B]
    perm = np.argsort(expert_flat, kind="stable")  # pairs grouped by expert
    counts = np.bincount(expert_flat, minlength=E)
    offs = np.concatenate(([0], np.cumsum(counts)))[:E]

    # ---- precision split: per expert, the NB largest-gate pairs run bf16;
    # the rest run fp8. NB is the smallest uniform bf16 pool size whose
    # predicted total error stays within budget. ----
    by_exp = [perm[offs[e] : offs[e] + counts[e]] for e in range(E)]
    g_by_exp = [np.sort(gate_flat[idx]) for idx in by_exp]
    cumg2 = [np.cumsum(g.astype(np.float64) ** 2) for g in g_by_exp]
    budget = TARGET_ERR**2 - BASE_ERR2

    def err2(NB):
        s = 0.0
        for e in range(E):
            nf = counts[e] - NB
            if nf > 0:
                s += cumg2[e][nf - 1]
        return ERR_COEF * s

    NB = int(min(counts))
    lo, hi = 1, int(min(counts))  # find smallest NB with err2 <= budget
    while lo < hi:
        mid = (lo + hi) // 2
        if err2(mid) <= budget:
            hi = mid
        else:
            lo = mid + 1
    NB = lo
    NB += -NB % 2  # keep bf16 x columns 4B-aligned
    NB = min(NB, int(min(counts)))
    nf_counts = counts - NB

    SF1, SF2, piecesF = _plan_segments(nf_counts)
    SFT = SF1 + SF2
    pred_err = float(np.sqrt(BASE_ERR2 + err2(NB)))

    # Per-expert class-local pair lists (token order): fp8 = smallest gates.
    f8_pairs, bf_pairs = [], []
    for e in range(E):
        idx = by_exp[e]
        ge = gate_flat[idx]
        nf = int(nf_counts[e])
        if nf > 0:
            thresh = np.partition(ge, nf - 1)[nf - 1]
            isf8 = ge < thresh
            need = nf - int(isf8.sum())  # break ties at the threshold
            at = np.nonzero(ge == thresh)[0]
            isf8[at[:need]] = True
        else:
            isf8 = np.zeros(len(idx), dtype=bool)
        f8_pairs.append(idx[isf8])
        bf_pairs.append(idx[~isf8])

    # Placement maps for the combine step.
    core_of_pair = np.empty(2 * B, dtype=np.int64)
    col_of_pair = np.empty(2 * B, dtype=np.int64)
    isf8_pair = np.zeros(2 * B, dtype=bool)

    xT = np.ascontiguousarray(x.T)  # [D, B] f32
    xT8 = xT.astype(_F8)
    f8_w1 = [None] * E
    f8_w2 = [None] * E
    bf_w1 = [None] * E
    bf_w2 = [None] * E

    in_maps = []
    for core in range(E):
        xf = np.zeros((D, SFT), dtype=_F8)
        in_map = {"xf": xf}
        # fp8 segments
        for seg, (e, start, ln) in enumerate(piecesF[core]):
            seg_off = 0 if seg == 0 else SF1
            if ln:
                pidx = f8_pairs[e][start : start + ln]
                toks = pidx // 2
                xf[:, seg_off : seg_off + ln] = xT8[:, toks]
                core_of_pair[pidx] = core
                col_of_pair[pidx] = seg_off + np.arange(ln)
                isf8_pair[pidx] = True
            if f8_w1[e] is None:
                f8_w1[e] = W1[e].astype(_F8)
                f8_w2[e] = W2[e].astype(_F8)
            s = "ab"[seg]
            in_map[f"w1f{s}"] = f8_w1[e]
            in_map[f"w2f{s}"] = f8_w2[e]
            in_map.setdefault("_b1f", []).append(
                np.ascontiguousarray(b1[e].reshape(MH, P).T)
            )
        # bf16 segment: core c <-> expert c
        e = core
        pidx = bf_pairs[e]
        assert len(pidx) == NB or len(pidx) == counts[e]
        xb = np.zeros((D, NB), dtype=_BF16)
        toks = pidx // 2
        xb[:, : len(pidx)] = xT[:, toks].astype(_BF16)
        core_of_pair[pidx] = core
        col_of_pair[pidx] = np.arange(len(pidx))
        if bf_w1[e] is None:
            bf_w1[e] = W1[e].astype(_BF16)
            bf_w2[e] = W2[e].astype(_BF16)
        in_map["xb"] = xb
        in_map["w1b"] = bf_w1[e]
        in_map["w2b"] = bf_w2[e]
        b1b_col = np.ascontiguousarray(b1[e].reshape(MH, P).T)
        in_map["b1all"] = np.ascontiguousarray(
            np.concatenate([b1b_col] + in_map.pop("_b1f"), axis=1)
        )
        in_maps.append(in_map)

    nc = _build_program(SF1, SF2, NB)

    from concourse.bass_utils import run_bass_kernel_spmd

    trace = os.environ.get("MOE_TRACE") == "1"
    kwargs = {}
    if trace:
        kwargs = dict(trace=True, trace_cores=list(range(E)))
    try:
        res = run_bass_kernel_spmd(nc, in_maps, core_ids=list(range(E)), **kwargs)
    except Exception:  # wedged accelerator: reset once and retry untraced
        try:
            import ctypes

            lib = ctypes.CDLL("/opt/axon/libaxon_pjrt.so")
            lib.axon_reset.restype = ctypes.c_int64
            lib.axon_reset()
        except OSError:
            pass
        res = run_bass_kernel_spmd(nc, in_maps, core_ids=list(range(E)))
    global LAST_RESULTS
    LAST_RESULTS = res
    LAST_RESULTS.pred_err = pred_err

    YF = np.stack([np.asarray(r["ytf"], dtype=np.float32) for r in res.results])
    YB = np.stack([np.asarray(r["ytb"], dtype=np.float32) for r in res.results])

    # Combine: pair p contributes gate_p * (y[:, col_p] + b2[e_p]) to token
    # p//2. Pairs of token b sit at flat positions 2b, 2b+1.
    cols = np.empty((2 * B, D), dtype=np.float32)
    m8 = isf8_pair
    cols[m8] = YF[core_of_pair[m8], :, col_of_pair[m8]]
    cols[~m8] = YB[core_of_pair[~m8], :, col_of_pair[~m8]]
    weighted = (cols + b2[expert_flat]) * gate_flat[:, None]
    out = weighted[0::2] + weighted[1::2]
    return np.ascontiguousarray(out, dtype=np.float32)
